# revision 19
# baseline (speedup 1.0000x reference)
"""Trainium2 Bass kernel for nn_Graphs (soft decision-graph probability propagation).

Reference math (G=4 graphs, B=128 batch, N=255 internal nodes, L=256 leaves,
F=512 features, J=8 jumps):
  b  = sigmoid(x @ W_g^T + bias_g)                  (per graph: B x N)
  M0 = softmax(M_left, axis=dest), M1 = softmax(M_right, axis=dest)
  q  = [b*(M1-M0)+M0 | leaf-identity]               (per (g,batch): 511x511)
  prob <- q @ prob, J times, starting from e0; return leaf probs.

Key restructure: q is never materialized. With u = prob[internal] and
v = b * u, one jump is
  prob_new = M0 @ u + (M1-M0) @ v   (+ leaf mass preserved)
where M0/Md are per-graph (511,255) matrices -> two small matmuls per jump.
Leaf rows only ever accumulate, so they live in a persistent PSUM
accumulator across all 8 jumps.

Sharding: 8 cores = (graph g = core//2) x (batch half h = core%2, 64 rows).
No cross-core communication. Host pre-transposes/pads inputs so the device
does zero transposes:
  - mlt/mrt (256,512): M^T with source node on partitions (pad row 255 = 0)
    and destination on free dim, leaf destinations shifted to cols 256..511
    (col 255 = -1e4 pad -> exp = 0). Softmax over dest = free-dim reduce.
  - wt (512,256): W_g^T (feature on partitions), pad node col 255 = 0.
  - xt (512,64): x_half^T.
  - biasp (256,1): bias_g padded.
Output per core: (256,64) leaf-major; host transposes/assembles to (B,L,G)
and applies the reference interval clamp.
"""

import numpy as np

G, B, N, L, F, J = 4, 128, 255, 256, 512, 8
BH = B // 2  # 64 batch rows per core
NCORES = 8
NEG = np.float32(-1e4)

_CACHE = {}


def _build_program():
    import concourse.mybir as mybir
    from concourse import bacc
    from concourse.tile import TileContext

    f32 = mybir.dt.float32
    f32r = mybir.dt.float32r  # single-pass fp32 matmul mode (4x faster at N>=256)
    AF = mybir.ActivationFunctionType
    AX = mybir.AxisListType

    def rmm(out, lhsT, rhs, **kw):
        nc.tensor.matmul(out, lhsT, rhs, **kw)

    # Bacc (not raw Bass): its compile() pass splits multi-wait instructions
    # into event semaphores, which the TRN2 ISA requires (1 wait/inst max).
    nc = bacc.Bacc(None)
    p_mlt = nc.declare_dram_parameter("mlt", [256, 512], f32, isOutput=False)
    p_mrt = nc.declare_dram_parameter("mrt", [256, 512], f32, isOutput=False)
    # wt (512,256) and xt (512,64) packed side by side: one DMA per K-tile so
    # each b-matmul's lhsT and rhs share a single DMA semaphore (the ISA
    # allows only one sync wait on a Matmult's LDWEIGHTS).
    p_wx = nc.declare_dram_parameter("wx", [512, 256 + BH], f32r, isOutput=False)
    p_bias = nc.declare_dram_parameter("biasp", [256, 1], f32, isOutput=False)
    p_out = nc.declare_dram_parameter("out", [BH, 256], f32, isOutput=True)

    with TileContext(nc) as tc:
        with (
            tc.tile_pool(name="consts", bufs=1) as consts,
            tc.tile_pool(name="work", bufs=2) as work,
            tc.tile_pool(name="state", bufs=2) as state,
            tc.tile_pool(name="psum", bufs=2, space="PSUM") as psum,
            tc.tile_pool(name="psum_acc", bufs=1, space="PSUM") as psum_acc,
        ):
            # ---- load inputs ----
            # wx first: the b-matmuls only need these, so PE can start early
            wx = [consts.tile([128, 256 + BH], f32r, tag=f"wx{k}", name=f"wx{k}") for k in range(4)]
            for k in range(4):
                nc.sync.dma_start(wx[k][:], p_wx[k * 128:(k + 1) * 128, :])
            bias = [consts.tile([128, 1], f32, tag=f"bias{t}", name=f"bias{t}") for t in range(2)]
            for t in range(2):
                nc.sync.dma_start(bias[t][:], p_bias[t * 128:(t + 1) * 128, :])
            # raw M^T tiles (f32), split in free-dim halves so exp can start
            # as soon as each half lands
            eraw = [consts.tile([128, 512], f32, tag=f"eraw{i}", name=f"eraw{i}") for i in range(4)]
            for i, p_m in ((0, p_mlt), (2, p_mrt)):
                for t in range(2):
                    for hh in range(2):
                        nc.sync.dma_start(
                            eraw[i + t][:, hh * 256:(hh + 1) * 256],
                            p_m[t * 128:(t + 1) * 128, hh * 256:(hh + 1) * 256],
                        )

            # ---- softmax, lazily normalized ----
            # el/er hold raw exp(M^T) (f32r); the softmax denominators r0/r1
            # are folded into the per-jump state scaling instead of scaling
            # the big matrices:  M0n@u + (M1n-M0n)@(b*u)
            #                  = E0@(r0*(1-b)*u) + E1@(r1*b*u)
            el = [consts.tile([128, 512], f32r, tag=f"el{t}", name=f"el{t}") for t in range(2)]
            er = [consts.tile([128, 512], f32r, tag=f"er{t}", name=f"er{t}") for t in range(2)]
            rec = []
            for i, mat in enumerate((el[0], el[1], er[0], er[1])):
                nc.scalar.activation(mat[:], eraw[i][:], AF.Exp)
                s = work.tile([128, 1], f32, tag="ssum", name="ssum")
                nc.vector.reduce_sum(s[:], mat[:], axis=AX.X)
                r = consts.tile([128, 1], f32, tag=f"srec{i}", name=f"srec{i}")
                nc.vector.reciprocal(r[:], s[:])
                rec.append(r)

            # ---- b = sigmoid(W @ x^T + bias) via exp (avoids a second ACT
            # table load), node-major (256,64); then fold softmax denominators:
            # c0 = r0*(1-b) = r0*eb/(1+eb), c1 = r1*b = r1/(1+eb), eb=exp(-logit)
            c0 = [consts.tile([128, BH], f32, tag=f"c0{t}", name=f"c0{t}") for t in range(2)]
            c1 = [consts.tile([128, BH], f32, tag=f"c1{t}", name=f"c1{t}") for t in range(2)]
            for mh in range(2):
                pb = psum.tile([128, BH], f32, tag="pb", name="pb")
                for k in range(4):
                    rmm(
                        pb[:], wx[k][:, mh * 128:(mh + 1) * 128],
                        wx[k][:, 256:256 + BH],
                        start=(k == 0), stop=(k == 3),
                    )
                eb = work.tile([128, BH], f32, tag="eb", name="eb")
                nc.scalar.activation(eb[:], pb[:], AF.Exp, bias=bias[mh][:], scale=-1.0)
                den = work.tile([128, BH], f32, tag="den", name="den")
                nc.vector.tensor_scalar_add(den[:], eb[:], 1.0)
                sig = work.tile([128, BH], f32, tag="sig", name="sig")
                nc.vector.reciprocal(sig[:], den[:])
                nc.vector.tensor_scalar_mul(c1[mh][:], sig[:], rec[2 + mh][:])
                nc.vector.tensor_mul(sig[:], sig[:], eb[:])
                nc.vector.tensor_scalar_mul(c0[mh][:], sig[:], rec[mh][:])

            # ---- jump loop ----
            # one-hot init built in f32 (memset can't encode f32r), rounded
            # into the f32r state via scalar copies
            z = [state.tile([128, BH], f32, tag=f"z{t}", name=f"z{t}") for t in range(2)]
            nc.vector.memset(z[0][:], 0.0)
            nc.vector.memset(z[1][:], 0.0)
            nc.vector.memset(z[0][0:1, :], 1.0)
            u = [state.tile([128, BH], f32r, tag=f"u{t}", name=f"u{t}") for t in range(2)]
            for t in range(2):
                nc.scalar.copy(u[t][:], z[t][:])
            # leaf accumulator: batch-major (64,256), one PSUM bank, N=256
            # moving dim keeps fp32r at 1 cycle/row.
            pleaf = psum_acc.tile([BH, 256], f32, tag="pl", name="pl")

            for j in range(J):
                up = [state.tile([128, BH], f32r, tag=f"up{t}", name=f"up{t}") for t in range(2)]
                v = [state.tile([128, BH], f32r, tag=f"v{t}", name=f"v{t}") for t in range(2)]
                for t in range(2):
                    nc.vector.tensor_mul(up[t][:], c0[t][:], u[t][:])
                    nc.vector.tensor_mul(v[t][:], c1[t][:], u[t][:])
                # internal-node block: node-major, fresh psum per jump.
                # Skipped on the last jump (u_J is never read).
                if j < J - 1:
                    pq = [psum.tile([128, BH], f32, tag=f"pq{mt}", name=f"pq{mt}") for mt in range(2)]
                    for mt in range(2):
                        ms = slice(mt * 128, (mt + 1) * 128)
                        rmm(pq[mt][:], el[0][:, ms], up[0][:], start=True, stop=False)
                        rmm(pq[mt][:], el[1][:, ms], up[1][:], start=False, stop=False)
                        rmm(pq[mt][:], er[0][:, ms], v[0][:], start=False, stop=False)
                        rmm(pq[mt][:], er[1][:, ms], v[1][:], start=False, stop=True)
                # leaf block: batch-major (lhsT = state), accumulated in PSUM
                # across all jumps
                first = j == 0
                last = j == J - 1
                rmm(pleaf[:], up[0][:], el[0][:, 256:512], start=first, stop=False)
                rmm(pleaf[:], up[1][:], el[1][:, 256:512], start=False, stop=False)
                rmm(pleaf[:], v[0][:], er[0][:, 256:512], start=False, stop=False)
                rmm(pleaf[:], v[1][:], er[1][:, 256:512], start=False, stop=last)
                if j < J - 1:
                    nu = [state.tile([128, BH], f32r, tag=f"u{t}", name=f"u{t}") for t in range(2)]
                    for t in range(2):
                        nc.scalar.copy(nu[t][:], pq[t][:])
                    u = nu

            # ---- output ----
            o = work.tile([BH, 256], f32, tag="o", name="o")
            nc.vector.tensor_copy(o[:], pleaf[:])
            nc.sync.dma_start(p_out[:, :], o[:])

    nc.finalize()
    return nc


def _get_program():
    if "nc" not in _CACHE:
        _CACHE["nc"] = _build_program()
    return _CACHE["nc"]


def _prep_inputs(x, W, bias, M_left, M_right):
    """Host-side shard + layout prep. Core c -> graph c//2, batch half c%2."""
    in_maps = []
    mlt_g, mrt_g, wt_g, bias_g = [], [], [], []
    for g in range(G):
        mlt = np.zeros((256, 512), np.float32)
        mrt = np.zeros((256, 512), np.float32)
        tl = np.ascontiguousarray(M_left[g].T)   # (255, 511)
        tr = np.ascontiguousarray(M_right[g].T)
        for dst, src in ((mlt, tl), (mrt, tr)):
            dst[0:255, 0:255] = src[:, 0:255]
            dst[0:255, 256:512] = src[:, 255:511]
            dst[0:255, 255] = NEG
        mlt_g.append(mlt)
        mrt_g.append(mrt)
        wt = np.zeros((512, 256), np.float32)
        wt[:, 0:255] = W[g].T
        wt_g.append(wt)
        # negated: the device computes exp(-(logit)) as exp(psum*-1 + bias_ap),
        # so bias_ap must carry -bias
        bp = np.zeros((256, 1), np.float32)
        bp[0:255, 0] = -bias[g]
        bias_g.append(bp)
    xt_h = [np.ascontiguousarray(x[h * BH:(h + 1) * BH].T) for h in range(2)]
    for c in range(NCORES):
        g, h = c // 2, c % 2
        wx = np.ascontiguousarray(np.concatenate([wt_g[g], xt_h[h]], axis=1))
        in_maps.append({
            "mlt": mlt_g[g], "mrt": mrt_g[g], "wx": wx, "biasp": bias_g[g],
        })
    return in_maps


def _assemble(results):
    eps = np.float32(1e-5)
    ret = np.empty((B, L, G), np.float32)
    for c in range(NCORES):
        g, h = c // 2, c % 2
        ret[h * BH:(h + 1) * BH, :, g] = results[c]["out"]
    ret = np.where(ret > 0.0, ret, eps)
    ret = np.where(ret < 1.0, ret, np.float32(1.0) - eps)
    return ret.astype(np.float32)


def run_on_device(in_maps, trace=False, **kw):
    from concourse.bass_utils import run_bass_kernel_spmd
    nc = _get_program()
    return run_bass_kernel_spmd(nc, in_maps, list(range(NCORES)), trace=trace, **kw)


def kernel(x, W, bias, M_left, M_right):
    in_maps = _prep_inputs(
        np.asarray(x, np.float32), np.asarray(W, np.float32),
        np.asarray(bias, np.float32), np.asarray(M_left, np.float32),
        np.asarray(M_right, np.float32),
    )
    res = run_on_device(in_maps)
    return _assemble(res.results)
